# revision 18
# baseline (speedup 1.0000x reference)
"""Trainium2 Bass kernel for nn_Attention_70136815943694.

Attention with the reference's source bug preserved (K uses the V
projection). x:[2,2048,1024], 16 heads x 64 dim. Sharded over 8
NeuronCores as (batch x head-group): core c handles batch c//4 and
heads [4*(c%4) .. 4*(c%4)+3]. Each core's output slice is independent,
so there are no collectives; the host shards inputs and reassembles.

Per-core device pipeline (d-major layouts):
  QT = wqT.T @ xT (+bq)     [256, 2048] bf16   (DVE evac w/ bias)
  KVT = wvT.T @ xT (+bv)    [256, 2048] bf16
  V   = PE-transpose of KVT chunks (bias included), + ones column
  per head-pair p, s1 quarter q (512 wide), s2 chunk j (128):
    scores: two K=64 matmuls row-packed via tile_position (0,0)/(64,0)
    PT = exp(0.125*scores) on ACT, one [128,1024] op for both heads
    atH += [V_h | 1].T @ PT_h   [65, 512] psum, row 64 = softmax denom
  epilogue: evac at->sbuf, DVE fast reciprocal of the denom row,
  gpsimd partition_broadcast, DVE mul, DMA out.

Input DMAs are split across all three DGE queues (sync + scalar HWDGE,
gpsimd SWDGE) because each HWDGE dma_start costs ~565ns of sequencer
issue time: weights ride gpsimd, xT column-halves ride sync (k0-3) and
scalar (k4-7) in deadline order, so the first scores' dependencies
(w m0 halves + xT cols 0:1024) land in ~4us instead of ~16.
"""
import numpy as np
import ml_dtypes

B = 2
S = 2048
D = 1024
NH = 16
HD = 64
N_CORES = 8
HEADS_PER_CORE = 4
DPC = HEADS_PER_CORE * HD  # 256 projection rows per core
P = 128
KC = D // P  # 8 contraction chunks
SC = S // P  # 16 s2 chunks
SQ = 512  # s1 quarter width
NSQ = S // SQ  # 4

_NC_CACHE = {}


def build_nc():
    if "nc" in _NC_CACHE:
        return _NC_CACHE["nc"]
    import concourse.bass as bass
    import concourse.mybir as mybir
    import concourse.tile as tile
    from concourse import bacc
    from concourse.masks import make_identity

    BF16 = mybir.dt.bfloat16
    F32 = mybir.dt.float32
    Act = mybir.ActivationFunctionType
    ts = bass.ts

    nc = bacc.Bacc(None, target_bir_lowering=False, debug=False)
    xT_d = nc.declare_dram_parameter("xT", [P, KC, S], BF16, isOutput=False)
    # w layout: [partition(k-row), half(m), qv, k-chunk, 128 m-cols]
    w_d = nc.declare_dram_parameter("w", [P, 2, 2, KC, P], BF16, isOutput=False)
    b_d = nc.declare_dram_parameter("b", [P, 2, 2], F32, isOutput=False)
    out_d = nc.declare_dram_parameter("out", [DPC, S], F32, isOutput=True)

    with tile.TileContext(nc) as tc:
        with (
            tc.tile_pool(name="persist", bufs=1) as persist,
            tc.tile_pool(name="pt", bufs=8) as pt_pool,
            tc.tile_pool(name="epi", bufs=2) as epi_pool,
        ):
            # warm the ACT exp table set at t~0 so the one-time table load
            # overlaps the input DMAs
            warm = persist.tile([1, 8], F32, tag="warm")
            nc.vector.memset(warm[:], 0.0)
            nc.scalar.activation(warm[:], warm[:], Act.Exp, scale=1.0)

            ident = persist.tile([P, P], BF16, tag="ident")
            make_identity(nc, ident[:])

            # ---- input loads. DMA-write dependencies resolve at tile
            # granularity (a reader waits for the LAST DMA into the tile),
            # so every DMA gets its own tile: 16 xt half-tiles, 2 weight
            # half-tiles. Deadline-ordered across the three queues.
            # xq_t[q][h]: xT quarter q, k-chunks 4h..4h+3 — one tile per DMA
            # (readers wait for the LAST DMA into a tile, and each DMA has
            # ~1.5-2us of fixed round-trip cost, so batch 4 k-chunks per
            # transfer and split by deadline: quarter 0 first).
            xq_t = [
                [
                    persist.tile([P, 4, SQ], BF16, name=f"xq{q}_{h}",
                                 tag=f"xq{q}_{h}")
                    for h in range(2)
                ]
                for q in range(NSQ)
            ]
            # weights m0 split q/v across the two HW queues so each queue's
            # critical prefix is ~768KB; m1 + biases ride gpsimd SWDGE.
            wm0_t = [
                persist.tile([P, KC, P], BF16, name=f"w0{qv}", tag=f"w0{qv}")
                for qv in range(2)
            ]
            w1_sb = persist.tile([P, 2, KC, P], BF16, name="w1", tag="w1")
            b_sb = persist.tile([P, 2, 2], F32, name="b", tag="b")

            # The DMA engines round-robin across ALL outstanding transfers,
            # so issue order alone cannot prioritize: only the first-exp
            # critical set (weights m0 + xT quarters 0-1, 2.5MB) is issued
            # up front. The rest (2.5MB) is held back behind a gpsimd copy
            # that depends on the prologue's KV evac, so it only starts
            # competing for bandwidth once the critical set has landed.
            nc.sync.dma_start(wm0_t[0][:], w_d[:, 0, 0])
            nc.scalar.dma_start(wm0_t[1][:], w_d[:, 0, 1])
            nc.gpsimd.dma_start(b_sb[:], b_d[:])
            for q in range(2):
                nc.sync.dma_start(xq_t[q][0][:], xT_d[:, 0:4, ts(q, SQ)])
                nc.scalar.dma_start(xq_t[q][1][:], xT_d[:, 4:8, ts(q, SQ)])

            KORDER = (0, 4, 1, 5, 2, 6, 3, 7)

            def xt(k, nq):
                return xq_t[nq][k // 4][:, k % 4, :]

            def wq(k, m):
                return (wm0_t[0] if m == 0 else w1_sb[:, 0])[:, k, :]

            def wv(k, m):
                return (wm0_t[1] if m == 0 else w1_sb[:, 1])[:, k, :]

            def bias(m, qv):
                return b_sb[:, m, qv : qv + 1]

            qT_sb = [
                persist.tile([P, S], BF16, name=f"qT{m}", tag=f"qT{m}")
                for m in range(2)
            ]
            kvT_sb = [
                persist.tile([P, S], BF16, name=f"kvT{m}", tag=f"kvT{m}")
                for m in range(2)
            ]
            # v_sb[2p+hl]: [128, SC, 65]; [:, j, 0:64] = V chunk j for head
            # 2p+hl, col 64 = 1 (softmax denominator rides the attnT matmul)
            v_sb = [
                persist.tile([P, SC, HD + 1], BF16, name=f"v{h}", tag=f"v{h}")
                for h in range(4)
            ]
            for h in range(4):
                nc.vector.memset(v_sb[h][:, :, HD : HD + 1], 1.0)

            def proj512(qv, dst, m, c0, psum_pool, stepped):
                """One 512-col slice [c0:c0+512] of projection (qv, m)."""
                w_of = wq if qv == 0 else wv
                ps = psum_pool.tile([P, 512], F32, tag="mi", name="pp")
                nq = c0 // 512
                for i, k in enumerate(KORDER):
                    nc.tensor.matmul(
                        ps[:],
                        w_of(k, m)[:],
                        xt(k, nq),
                        start=(i == 0),
                        stop=(i == KC - 1),
                    )
                    if stepped and i % 2 == 1:
                        yield
                nc.vector.tensor_scalar_add(
                    dst[:, ts(nq, 512)], ps[:], bias(m, qv)[:]
                )
                if stepped:
                    yield

            def vtrans_steps(p, psum_pool, j0, j1, per_step=2):
                """PE-transpose KVT chunks into natural-layout V tiles."""
                n = 0
                for j in range(j0, j1):
                    pst = psum_pool.tile(
                        [P, P], BF16, tag="mi", name="vt",
                        padded_shape=[P, 1024],
                    )
                    nc.tensor.transpose(
                        pst[:], kvT_sb[p][:, ts(j, P)], ident[:]
                    )
                    for hl in range(2):
                        nc.vector.tensor_copy(
                            v_sb[2 * p + hl][:, j, 0:HD], pst[:, ts(hl, HD)]
                        )
                    n += 1
                    if n % per_step == 0:
                        yield

            # ---- prologue: minimum before exps can flow: qT m0 cols 0:512
            # and KVT m0 cols 0:1024 (scores lhsT for j<8 + first V
            # transposes). Warm-up ident matmuls lift the PE clock gate and
            # p-state while the input DMAs land.
            with tc.tile_pool(name="psum_pro", bufs=4, space="PSUM") as psum_pro:
                wps = psum_pro.tile([P, 512], F32, tag="warm", name="wps")
                for i in range(10):
                    nc.tensor.matmul(
                        wps[:, 0:P], ident[:], ident[:], start=True, stop=True
                    )

                ps_q = psum_pro.tile([P, 512], F32, tag="mi", name="ppq")
                ps_v0 = psum_pro.tile([P, 512], F32, tag="mi", name="ppv0")
                # k in DMA-arrival order; two dependency-free warm matmuls
                # between chunk pairs keep the PE clock/p-state up while the
                # next chunk's DMA lands.
                for i, k in enumerate(KORDER):
                    nc.tensor.matmul(
                        ps_v0[:],
                        wv(k, 0)[:],
                        xt(k, 0),
                        start=(i == 0),
                        stop=(i == KC - 1),
                    )
                    nc.tensor.matmul(
                        ps_q[:],
                        wq(k, 0)[:],
                        xt(k, 0),
                        start=(i == 0),
                        stop=(i == KC - 1),
                    )
                    if i < KC - 1:
                        for _ in range(2):
                            nc.tensor.matmul(
                                wps[:, 0:P], ident[:], ident[:],
                                start=True, stop=True,
                            )
                nc.vector.tensor_scalar_add(
                    kvT_sb[0][:, 0:512], ps_v0[:], bias(0, 1)[:]
                )
                nc.vector.tensor_scalar_add(qT_sb[0][:, 0:512], ps_q[:], bias(0, 0)[:])
                # read wps once so the warm-up matmuls aren't dead code
                nc.vector.tensor_copy(warm[:], wps[0:1, 0:8])

            # Second DMA wave, released only now: a one-element copy from
            # the freshly-evac'd kvT into each destination tile makes the
            # DMA issue wait (WAR) until the critical first wave has landed.
            late = [
                (nc.sync, xq_t[2][0], xq_t[2][0][0:1, 0, 0:1],
                 xT_d[:, 0:4, ts(2, SQ)]),
                (nc.scalar, xq_t[2][1], xq_t[2][1][0:1, 0, 0:1],
                 xT_d[:, 4:8, ts(2, SQ)]),
                (nc.sync, xq_t[3][0], xq_t[3][0][0:1, 0, 0:1],
                 xT_d[:, 0:4, ts(3, SQ)]),
                (nc.scalar, xq_t[3][1], xq_t[3][1][0:1, 0, 0:1],
                 xT_d[:, 4:8, ts(3, SQ)]),
                (nc.gpsimd, w1_sb, w1_sb[0:1, 0, 0, 0:1], w_d[:, 1]),
            ]
            for eng, dst_tile, gate_ap, src in late:
                nc.vector.tensor_copy(gate_ap, kvT_sb[0][0:1, 0:1])
                eng.dma_start(dst_tile[:], src)

            # ---- attention ---------------------------------------------------
            with (
                tc.tile_pool(name="psum_sc", bufs=2, space="PSUM") as psum_sc,
                tc.tile_pool(name="psum_at", bufs=2, space="PSUM") as psum_at,
                tc.tile_pool(name="psum_mi", bufs=2, space="PSUM") as psum_mi,
            ):
                # Preseed V chunks 0,1 (need only kvT[0][:,0:256]) so (0,0)'s
                # first attnT matmuls have emitted writers.
                vt00 = vtrans_steps(0, psum_mi, 0, 2)
                next(vt00)

                def adv(g, n=1):
                    for _ in range(n):
                        try:
                            next(g)
                        except StopIteration:
                            return

                def fill00():
                    """(0,0) filler, deadline-ordered: kv m0 quarter q must
                    complete before scores j=4q; V chunk j before attnT-j
                    (emitted at slot j+1); kv [512:1024] evac before j=4."""
                    kv1 = proj512(1, kvT_sb[0], 0, 512, psum_mi, True)
                    kv2 = proj512(1, kvT_sb[0], 0, 1024, psum_mi, True)
                    kv3 = proj512(1, kvT_sb[0], 0, 1536, psum_mi, True)
                    q1 = proj512(0, qT_sb[0], 0, 512, psum_mi, True)
                    vt = vtrans_steps(0, psum_mi, 2, 16, per_step=1)
                    # slot:      0      1      2      3
                    adv(kv1, 2); yield
                    adv(kv1, 2); adv(vt, 1); yield
                    adv(kv1, 1); adv(vt, 1); yield  # kv[512:1024] ready
                    adv(kv2, 2); adv(vt, 1); yield
                    # slot:      4      5      6      7
                    adv(kv2, 2); adv(vt, 1); yield
                    adv(kv2, 1); adv(vt, 1); yield  # kv[1024:1536] ready
                    adv(kv3, 2); adv(vt, 1); yield
                    adv(kv3, 2); adv(vt, 1); yield
                    # slot:      8      9     10     11
                    adv(kv3, 1); adv(vt, 1); yield  # kv[1536:2048] ready
                    adv(vt, 1); adv(q1, 1); yield
                    adv(vt, 1); adv(q1, 1); yield
                    adv(vt, 1); adv(q1, 1); yield
                    # slot:     12     13     14     15
                    adv(vt, 1); adv(q1, 1); yield
                    adv(vt, 1); adv(q1, 1); yield  # qT[512:1024] ready
                    adv(vt, 1); yield
                    adv(vt, 1); yield

                def sched(pattern, *gs):
                    """Deterministic filler: pattern[i] = generator index to
                    advance one step at slot i. Keeps every evac inside its
                    group and ahead of its readers (Tile deps follow trace
                    order, so an evac emitted after its reader is a race)."""
                    for idx in pattern:
                        adv(gs[idx], 1)
                        yield

                # Remaining fillers, explicit per-slot schedules chosen so
                # each proj512's evac lands before its first reader. All of
                # v p1 (16 chunks) is consumed by group (1,0)'s own j-scan,
                # so vtrans p1 chunk j must be emitted before (1,0)'s
                # attnT-j (slot j+1 of (1,0)).
                fillers = {
                    (0, 0): [fill00()],
                    (0, 1): [sched(
                        [0, 1] * 5,
                        proj512(0, qT_sb[0], 0, 1024, psum_mi, True),
                        proj512(1, kvT_sb[1], 1, 0, psum_mi, True),
                    )],
                    (0, 2): [sched(
                        [0, 1, 0, 1, 2, 0, 1, 0, 1, 2, 0, 1],
                        proj512(1, kvT_sb[1], 1, 512, psum_mi, True),
                        proj512(0, qT_sb[0], 0, 1536, psum_mi, True),
                        vtrans_steps(1, psum_mi, 0, 4),
                    )],
                    (0, 3): [sched(
                        [0, 1, 0, 1, 2, 0, 1, 0, 1, 2, 0, 1],
                        proj512(0, qT_sb[1], 1, 0, psum_mi, True),
                        proj512(1, kvT_sb[1], 1, 1024, psum_mi, True),
                        vtrans_steps(1, psum_mi, 4, 8),
                    )],
                    (1, 0): [sched(
                        [0, 0, 0, 0, 0, 1, 1, 2, 2, 2, 2, 2, 1, 1],
                        proj512(1, kvT_sb[1], 1, 1536, psum_mi, True),
                        vtrans_steps(1, psum_mi, 8, 16),
                        proj512(0, qT_sb[1], 1, 512, psum_mi, True),
                    )],
                    (1, 1): [sched(
                        [0] * 5,
                        proj512(0, qT_sb[1], 1, 1024, psum_mi, True),
                    )],
                    (1, 2): [sched(
                        [0] * 5,
                        proj512(0, qT_sb[1], 1, 1536, psum_mi, True),
                    )],
                }

                def emit_epilogue(p, q, at):
                    for hl in range(2):
                        head = 2 * p + hl
                        # single [65,512] evac (rows 0:64 numerator, row 64
                        # denom) releases the at psum tile quickly.
                        asb = epi_pool.tile([HD + 1, SQ], F32, tag="asb", name="asb")
                        nc.vector.tensor_copy(asb[:], at[hl][:])
                        # partition_broadcast reads partition 0: stage the
                        # denom row in a p0 tile, then take its reciprocal.
                        dr = epi_pool.tile([1, SQ], F32, tag="dr", name="dr")
                        nc.vector.tensor_copy(dr[:], asb[HD : HD + 1, :])
                        rc = epi_pool.tile([1, SQ], F32, tag="rc", name="rc")
                        nc.vector.reciprocal_approx_fast(rc[:], dr[:])
                        bc = epi_pool.tile([HD, SQ], F32, tag="bc", name="bc")
                        nc.gpsimd.partition_broadcast(bc[:], rc[:])
                        ot = epi_pool.tile([HD, SQ], F32, tag="ot", name="ot")
                        nc.vector.tensor_mul(ot[:], asb[0:HD, :], bc[:])
                        nc.sync.dma_start(out_d[ts(head, HD), ts(q, SQ)], ot[:])

                # Software-pipelined: attnT for slot i is emitted during slot
                # i+1 so the next group's scores/exp never sit behind the
                # previous group's last attnT in PE program order.
                slots = [(p, q, j) for p in range(2) for q in range(NSQ)
                         for j in range(SC)]
                gens = []
                at = None
                prev = None
                for p, q, j in slots:
                    if j == 0:
                        gens = fillers.get((p, q), []) + gens
                        at = [
                            psum_at.tile([HD + 1, SQ], F32, tag="at", name="at")
                            for _ in range(2)
                        ]
                    sc = psum_sc.tile([P, 1024], F32, tag="sc", name="sc")
                    for hl in range(2):
                        nc.tensor.matmul(
                            sc[:, ts(hl, SQ)],
                            kvT_sb[p][hl * HD : (hl + 1) * HD, ts(j, P)],
                            qT_sb[p][hl * HD : (hl + 1) * HD, ts(q, SQ)],
                            start=True,
                            stop=True,
                            tile_position=(hl * HD, 0),
                        )
                    pt = pt_pool.tile([P, 1024], BF16, tag="pt", name="pt")
                    nc.scalar.activation(pt[:], sc[:], Act.Exp, scale=0.125)
                    # filler work; one step per slot keeps PE bursts smaller
                    # than the exp time
                    if gens:
                        g = gens.pop(0)
                        try:
                            next(g)
                            gens.append(g)
                        except StopIteration:
                            pass
                    if prev is not None:
                        pp, pq, pj, pat, ppt = prev
                        for hl in range(2):
                            nc.tensor.matmul(
                                pat[hl][:],
                                v_sb[2 * pp + hl][:, pj, :],
                                ppt[:, ts(hl, SQ)],
                                start=(pj == 0),
                                stop=(pj == SC - 1),
                            )
                        if pj == SC - 1:
                            emit_epilogue(pp, pq, pat)
                    prev = (p, q, j, at, pt)
                # flush the last slot immediately (no 1-slot lag on the tail)
                p, q, j, at, pt = prev
                for hl in range(2):
                    nc.tensor.matmul(
                        at[hl][:],
                        v_sb[2 * p + hl][:, j, :],
                        pt[:, ts(hl, SQ)],
                        start=False,
                        stop=True,
                    )
                emit_epilogue(p, q, at)

    nc.compile()
    _NC_CACHE["nc"] = nc
    return nc


def shard_inputs(x, Wq, bq, Wv, bv):
    bf16 = ml_dtypes.bfloat16
    x = np.asarray(x, dtype=np.float32)
    Wq = np.asarray(Wq, dtype=np.float32)
    bq = np.asarray(bq, dtype=np.float32)
    Wv = np.asarray(Wv, dtype=np.float32)
    bv = np.asarray(bv, dtype=np.float32)
    in_maps = []
    # xT per batch: [P, KC, S] (k-chunk-major rows)
    xT = [
        np.ascontiguousarray(
            x[b].T.reshape(KC, P, S).transpose(1, 0, 2)
        ).astype(bf16)
        for b in range(B)
    ]
    for c in range(N_CORES):
        b, g = divmod(c, N_CORES // B)
        heads = [HEADS_PER_CORE * g + hl for hl in range(HEADS_PER_CORE)]
        perm = np.array([i * NH + h for h in heads for i in range(HD)])
        # [D(k-major rows: KC x P), DPC] -> [P, KC, DPC]
        wqT = Wq[perm, :].T.reshape(KC, P, DPC).transpose(1, 0, 2)
        wvT = Wv[perm, :].T.reshape(KC, P, DPC).transpose(1, 0, 2)
        # -> [P, half(m), qv, KC, 128]
        w = np.empty((P, 2, 2, KC, P), dtype=np.float32)
        for m in range(2):
            w[:, m, 0] = wqT[:, :, m * P : (m + 1) * P]
            w[:, m, 1] = wvT[:, :, m * P : (m + 1) * P]
        bb = np.empty((P, 2, 2), dtype=np.float32)
        for m in range(2):
            bb[:, m, 0] = bq[perm][m * P : (m + 1) * P]
            bb[:, m, 1] = bv[perm][m * P : (m + 1) * P]
        in_maps.append(
            {
                "xT": xT[b],
                "w": np.ascontiguousarray(w).astype(bf16),
                "b": np.ascontiguousarray(bb),
            }
        )
    return in_maps


def assemble(results):
    out = np.empty((B, S, D), dtype=np.float32)
    for c in range(N_CORES):
        b, g = divmod(c, N_CORES // B)
        out[b][:, g * DPC : (g + 1) * DPC] = results[c]["out"].T
    return out


def kernel(x, Wq, bq, Wv, bv):
    from concourse.bass_utils import run_bass_kernel_spmd

    nc = build_nc()
    in_maps = shard_inputs(x, Wq, bq, Wv, bv)
    res = run_bass_kernel_spmd(nc, in_maps, core_ids=list(range(N_CORES)))
    return assemble(res.results)


if __name__ == "__main__":
    rng = np.random.default_rng(0)
    inputs = {
        "x": rng.standard_normal((B, S, D), dtype=np.float32),
        "Wq": (rng.standard_normal((D, D), dtype=np.float32) / 32.0),
        "bq": rng.standard_normal(D, dtype=np.float32) * 0.02,
        "Wv": (rng.standard_normal((D, D), dtype=np.float32) / 32.0),
        "bv": rng.standard_normal(D, dtype=np.float32) * 0.02,
    }
    out = kernel(**inputs)
    print("kernel ran, out shape:", out.shape)
